# revision 33
# baseline (speedup 1.0000x reference)
"""Trainium2 Bass kernel for nn_DAGModel (gnn_message_passing).

Strategy (data-parallel over batch, 8 b's per core):
- node_vecs live in DRAM as a bf16 table `nv[token, b8, h128]` (2KB rows).
- Parent gathers use GPSIMD dma_gather(transpose=True) in PREPARE_ONLY
  mode + trigger_dma: the gpsimd only generates descriptors (~1.1us) and
  the 2KB-row transfers run asynchronously on the DMA engines, so the
  Pool engine is no longer serialized on gather transfer time.
- Nodes of each depth are reordered host-side by (has-parent-in-previous-
  depth, parent-count desc). Chunks whose parents all come from older
  depths bound their gather source AP below the previous depth's slab, so
  those gathers overlap the previous depth's tail compute/writeback.
- The parent-slot sum accumulates IN PLACE into the slot-0 segment of the
  gathered tile (bf16 adds on DVE), which doubles as the MLP rhs `pv`.
- The 2-layer MLP runs in bf16 on the PE (f32 PSUM accumulate); ReLU+b1
  and +b2 ride the Scalar engine's activation; the residual +pv is a DVE
  bf16 add.
- Output projection out[t] = nv[t]·Wout[t] is computed feature-major:
  m = nvn * woutT elementwise (DVE, bf16), then a ones-vector matmul
  reduces over partitions into PSUM rows (per b-pair at partition 32*bp).
- new vecs are PE-transposed (bf16) back to row-major and DMA'd to the
  next depth's token rows.
"""

import numpy as np
import ml_dtypes

BF16 = ml_dtypes.bfloat16

# Full-problem dims (hardcoded per contract).
B, H, E = 64, 128, 128
D_FULL, P_FULL, MP = 20, 1000, 8
NCORES, BL = 8, 8
BCAP = 512  # SWDGE ring is ~512 descs/dir
LAST_RESULTS = None


# ---------------------------------------------------------------------------
# workaround: this walrus build rejects >1 sync-wait on a CTRL (Drain) inst.
def _install_tilefix():
    import concourse.tile as tile_mod
    from concourse.vector_clock import ScopedClock, VectorClock

    if getattr(tile_mod.TileContext, "_drain_split_installed", False):
        return

    def _split_drain_and_barrier(self, tick_clock, wait_clock):
        gc = tick_clock.global_clock
        ticks = list(gc)
        nz = [(i, t) for i, t in enumerate(ticks) if t > 0]
        if nz:
            for i, t in nz:
                vec = [0] * len(ticks)
                vec[i] = t
                d = self.nc.sync.drain()
                wait_clock.add_sem_waits(
                    d.ins, ScopedClock({None: VectorClock(vec)})
                )
        else:
            d = self.nc.sync.drain()
            wait_clock.add_sem_waits(d.ins, ScopedClock({None: gc}))
        self.nc.all_engine_barrier()
        assert self.sems is not None
        popped = self.nc._tile_sem_poison_stack.pop()
        assert popped is self._sem_poison
        self.nc.clear_and_free_semaphores(list(self.sems.allocated().values()))
        self.nc.all_engine_barrier()

    tile_mod.TileContext._drain_and_barrier = _split_drain_and_barrier
    tile_mod.TileContext._drain_split_installed = True


def _install_usersync_prep():
    """Route gen_mode==1 SWDGE gather preps onto their ENGINE proc
    (user-synced protocol) instead of a DMASW lane: Tile's DMASW-lane
    path for preps emits a pre-bumped doorbell + mismatched completion
    sem and deadlocks/races on this build. With the engine tick also
    registered in prep_eng_ticks, pass 2 gates trigger_dma on desc-gen
    completion; data completion is via the caller's sem= semaphore and
    explicit _wait_ge on consumer instructions."""
    import concourse.tile_sem_assignment as tsa
    import concourse.mybir as mybir

    if getattr(tsa.TileClockTick, "_usersync_prep_installed", False):
        return
    orig = tsa.TileClockTick._assign_tick

    def patched(self, inst):
        if getattr(inst, "gen_mode", 0) == 1 and isinstance(
            inst, (mybir.InstDMAGatherAnt, mybir.InstDMAScatterAddAnt)
        ):
            eng_proc_idx = (
                tsa.ENGINE_SEQUENCER_TO_IDX
                if inst.is_sequencer_only()
                else tsa.ENGINE_TO_IDX
            )[inst.engine]
            tick = self.global_clock.advance(eng_proc_idx)
            inst.bass_scheduled_tick = tick
            inst.bass_scheduled_proc = eng_proc_idx
            inst.bass_scheduled_scope = self.scope_name
            self._proc_insts[self.root_scope_name][eng_proc_idx].append(inst)
            self.tc.prep_eng_ticks[inst.name] = (eng_proc_idx, tick)
            self._prep_eng_names[self.root_scope_name].append(inst.name)
            return
        return orig(self, inst)

    tsa.TileClockTick._assign_tick = patched
    tsa.TileClockTick._usersync_prep_installed = True


# ---------------------------------------------------------------------------
def _wrap_idx(seq):
    """int16 index layout for dma_gather: position i -> [i%16, i//16],
    replicated across the 8 groups of 16 partitions."""
    a = np.asarray(seq, np.int16)
    L = len(a)
    assert L % 16 == 0
    a16 = a.reshape(L // 16, 16).T  # [16, L/16]
    return np.ascontiguousarray(np.tile(a16, (8, 1)))  # [128, L/16]


def _prepare(inputs, D, P, CH, NCH):
    """Host-side index preprocessing + weight/emb staging (layout only)."""
    PP = CH * NCH
    node_indices = np.asarray(inputs["node_indices"])
    parent_indices = np.asarray(inputs["parent_indices"])
    k = (parent_indices > 0).sum(-1)  # [D, P]
    recent = np.zeros((D, P), bool)
    for d in range(1, D):
        recent[d] = (parent_indices[d] >= 2 + (d - 1) * P).any(-1)

    remap = np.zeros(2 + D * P, np.int64)
    remap[1] = 1
    perms = []
    for d in range(D):
        # old-parent nodes first, then by parent count desc (prefix trick)
        perm = np.lexsort((-k[d], recent[d]))
        perms.append(perm)
        remap[2 + d * P + perm] = 2 + d * PP + np.arange(P)

    chunk_meta = []  # [d][c] -> dict(blens, adds, bound, w16)
    pidx_chunks = {}
    MAXW16 = 0
    MAXB = 0
    for d in range(D):
        perm = perms[d]
        row = []
        for c in range(NCH):
            lo = c * CH
            hi = min(lo + CH, P)
            nodes = perm[lo:hi]
            nreal = len(nodes)
            kc = k[d][nodes]
            rc = recent[d][nodes]
            og = int((~rc).sum())
            if d == 0:
                bound = 2
            elif og == nreal:
                bound = 2 + (d - 1) * PP
            else:
                bound = 2 + d * PP

            # segments: (col_start, idx_array); slot0 first (becomes pv)
            segs = [(0, np.pad(remap[parent_indices[d, nodes, 0]],
                               (0, CH - nreal)))]
            for j in range(1, MP):
                mo = int((kc[:og] > j).sum())
                if mo:
                    segs.append((0, remap[parent_indices[d, nodes[:mo], j]]))
                mr = int((kc[og:] > j).sum())
                if mr:
                    segs.append(
                        (og, remap[parent_indices[d, nodes[og:og + mr], j]]))

            # pack segments into <=BCAP bundles; record add ops
            blens, adds = [], []
            wrapped = []
            cur, cur_len = [], 0

            def close():
                nonlocal cur, cur_len
                if not cur_len:
                    return
                idx = np.concatenate(cur)
                L = (len(idx) + 127) // 128 * 128
                idx = np.pad(idx, (0, L - len(idx)))
                wrapped.append(_wrap_idx(idx))
                blens.append(L)
                cur, cur_len = [], 0

            for si, (cs, arr) in enumerate(segs):
                L = len(arr)
                if cur_len + L > BCAP and cur_len > 0:
                    close()
                if si > 0:
                    adds.append((len(blens), cur_len, cs, L))
                cur.append(arr)
                cur_len += L
            close()

            w16 = sum(L // 16 for L in blens)
            pidx_chunks[(d, c)] = np.concatenate(wrapped, axis=1)
            MAXW16 = max(MAXW16, w16)
            MAXB = max(MAXB, len(blens))
            row.append({"blens": blens, "adds": adds, "bound": bound,
                        "w16": w16})
        chunk_meta.append(row)

    pidx_np = np.zeros((D, NCH, 128, MAXW16), np.int16)
    for (d, c), w in pidx_chunks.items():
        pidx_np[d, c, :, : w.shape[1]] = w

    emb = np.asarray(inputs["emb_table"], np.float32)
    Wout = np.asarray(inputs["Wout"], np.float32)
    W1 = np.asarray(inputs["W1"], np.float32)
    W2 = np.asarray(inputs["W2"], np.float32)

    # per-chunk feature-major emb rows + Wout rows: [D, NCH, h, 2, CH]
    newt = np.zeros((D, NCH, H, 2, CH), np.float32)
    for d in range(D):
        for c in range(NCH):
            lo = c * CH
            hi = min(lo + CH, P)
            nodes = perms[d][lo:hi]
            newt[d, c, :, 0, : hi - lo] = emb[node_indices[d][nodes]].T
            newt[d, c, :, 1, : hi - lo] = Wout[1 + d * P + nodes].T

    wcat = np.zeros((128, 5, 128), np.float32)
    wcat[:, 0] = W1[:, :H].T
    wcat[:, 1] = W1[:, H:].T
    wcat[:, 2] = W2.T
    wcat[:, 3] = np.eye(128)
    wcat[:, 4, 0] = 1.0  # ones column for the partition-reduce matmul

    prep = {
        "meta": chunk_meta,
        "perms": perms,
        "maxw16": MAXW16,
        "pidx": pidx_np,
        "newt": np.ascontiguousarray(newt.astype(BF16)),
        "wcat": np.ascontiguousarray(wcat.astype(BF16)),
        "b1": np.asarray(inputs["b1"], np.float32).reshape(128, 1),
        "b2": np.asarray(inputs["b2"], np.float32).reshape(128, 1),
    }
    return prep


def _build(prep, D, P, CH, NCH):
    """Trace the Bass/Tile kernel. Returns a finalized Bacc."""
    import os
    PREP = os.environ.get("KPREP", "1") == "1"
    PREPN = int(os.environ.get("KPREPN", "99999"))
    _install_tilefix()
    if PREP:
        _install_usersync_prep()
    from contextlib import ExitStack

    import concourse.bacc as bacc
    import concourse.mybir as mybir
    from concourse.tile import TileContext

    PP = CH * NCH
    TOK = 2 + D * PP
    ROW = BL * H  # nv row elems (bf16)
    KB = CH // 128
    f32 = mybir.dt.float32
    bf16 = mybir.dt.bfloat16
    i16 = mybir.dt.int16
    AF = mybir.ActivationFunctionType

    nc = bacc.Bacc("TRN2", target_bir_lowering=False, debug=False)

    nv = nc.dram_tensor("nv", [TOK, ROW], bf16, kind="Internal")
    nvinit = nc.dram_tensor("nvinit", [2, ROW], bf16, kind="ExternalInput")
    pidx_in = nc.dram_tensor(
        "pidx", list(prep["pidx"].shape), i16, kind="ExternalInput"
    )
    newt_in = nc.dram_tensor(
        "newt", [D, NCH, 128, 2, CH], bf16, kind="ExternalInput"
    )
    wcat_in = nc.dram_tensor("wcat", [128, 5, 128], bf16, kind="ExternalInput")
    b1_in = nc.dram_tensor("b1c", [128, 1], f32, kind="ExternalInput")
    b2_in = nc.dram_tensor("b2c", [128, 1], f32, kind="ExternalInput")
    outd = nc.dram_tensor("outd", [D, NCH, 4, 2, CH], f32,
                          kind="ExternalOutput")

    meta = prep["meta"]

    with TileContext(nc) as tc, ExitStack() as ctx:
        const = ctx.enter_context(tc.tile_pool(name="const", bufs=1))
        pidx_pool = ctx.enter_context(tc.tile_pool(name="pidx", bufs=3))
        stag_pool = ctx.enter_context(tc.tile_pool(name="stag", bufs=3))
        newt_pool = ctx.enter_context(tc.tile_pool(name="newt", bufs=3))
        h1_pool = ctx.enter_context(tc.tile_pool(name="h1", bufs=4))
        nvn_pool = ctx.enter_context(tc.tile_pool(name="nvn", bufs=10))
        m_pool = ctx.enter_context(tc.tile_pool(name="m", bufs=2))
        nvrm_pool = ctx.enter_context(tc.tile_pool(name="nvrm", bufs=4))
        outsb_pool = ctx.enter_context(tc.tile_pool(name="outsb", bufs=2))
        psmm = ctx.enter_context(tc.tile_pool(name="psmm", bufs=2, space="PSUM"))
        psm2 = ctx.enter_context(tc.tile_pool(name="psm2", bufs=2, space="PSUM"))
        pstp = ctx.enter_context(tc.tile_pool(name="pstp", bufs=2, space="PSUM"))
        pso = ctx.enter_context(tc.tile_pool(name="pso", bufs=1, space="PSUM"))

        # rotation depth must be >= stag pool bufs+1 so same-sem chunks
        # can never have gathers in flight concurrently
        dma_sems = [nc.alloc_semaphore(f"swdma{i}") for i in range(4)]
        gath_cnt = [0, 0, 0, 0]

        wcat = const.tile([128, 5, 128], bf16)
        nc.sync.dma_start(out=wcat[:], in_=wcat_in[:, :, :])
        w1at = wcat[:, 0, :]
        w1bt = wcat[:, 1, :]
        w2t = wcat[:, 2, :]
        identb = wcat[:, 3, :]
        ones_col = wcat[:, 4, 0:1]
        b1 = const.tile([128, 1], f32)
        nc.sync.dma_start(out=b1[:], in_=b1_in[:, :])
        b2 = const.tile([128, 1], f32)
        nc.sync.dma_start(out=b2[:], in_=b2_in[:, :])

        # init nv rows 0..1 (zero pad row + root = per-b embedding)
        import bass_rust as _br
        _DI = _br.DependencyInfo(sync=True, no_sync=False)
        nv_writes = {}  # depth -> [dma inst names]
        nvi = const.tile([2, ROW], bf16)
        nc.sync.dma_start(out=nvi[:], in_=nvinit[:, :])
        winit = nc.sync.dma_start(out=nv[0:2, :], in_=nvi[:])
        nv_writes[-1] = [winit.ins.name]

        nchunk = 0
        for d in range(D):
            for c in range(NCH):
                cprep = PREP and nchunk < PREPN
                par = nchunk % 4
                dma_sem = dma_sems[par]
                nchunk += 1
                md = meta[d][c]
                blens, adds, bound = md["blens"], md["adds"], md["bound"]

                pidx_sb = pidx_pool.tile([128, md["w16"]], i16)
                nc.sync.dma_start(
                    out=pidx_sb[:], in_=pidx_in[d, c, :, : md["w16"]]
                )
                newt_sb = newt_pool.tile([128, 2, CH], bf16)
                nc.sync.dma_start(out=newt_sb[:], in_=newt_in[d, c])

                # source-writer sync deps go on the first PREP (the
                # trigger cannot carry >1 sem wait on this build)
                if d == 0:
                    wdeps = nv_writes[-1]
                elif bound == 2 + (d - 1) * PP:
                    wdeps = nv_writes.get(d - 2, nv_writes[-1])
                else:
                    wdeps = nv_writes[d - 1]
                stags = []
                off16 = 0
                for bi, L in enumerate(blens):
                    stag = stag_pool.tile([128, BL, L], bf16, tag=f"stag{bi}")
                    if cprep:
                        g = nc.gpsimd.dma_gather(
                            stag[:], nv[0:bound, :],
                            pidx_sb[:, off16 : off16 + L // 16],
                            num_idxs=L, num_idxs_reg=L,
                            elem_size=ROW, transpose=True,
                            prepare_only=True, sem=dma_sem,
                        )
                        for wn in wdeps:
                            g.ins.add_dependency(wn, _DI)
                        nc.gpsimd.trigger_dma(count=None)
                    else:
                        nc.gpsimd.dma_gather(
                            stag[:], nv[0:bound, :],
                            pidx_sb[:, off16 : off16 + L // 16],
                            num_idxs=L, num_idxs_reg=L,
                            elem_size=ROW, transpose=True,
                        )
                    off16 += L // 16
                    stags.append(stag)
                if cprep:
                    gath_cnt[par] += len(blens)
                gtarget = 16 * gath_cnt[par]

                # ---- parent-slot reduction in place into slot-0 segment
                pv = stags[0]
                for (bi, off, cs, L) in adds:
                    a = nc.vector.tensor_add(
                        pv[:, :, cs : cs + L],
                        pv[:, :, cs : cs + L],
                        stags[bi][:, :, off : off + L],
                    )
                    if cprep:
                        a._wait_ge(dma_sem, gtarget)

                # ---- MLP (bf16) over col pairs (2 b's x CH = 512 cols)
                ne_b = newt_sb[:, 0:1, :]
                nvns = []
                for bp in range(BL // 2):
                    pv2 = pv[:, 2 * bp : 2 * bp + 2, 0:CH]
                    h1p = psmm.tile([128, 2, CH], f32, tag="h1p")
                    mm = nc.tensor.matmul(
                        h1p[:], lhsT=w1at, rhs=pv2, start=True, stop=False
                    )
                    if cprep:
                        mm._wait_ge(dma_sem, gtarget)
                    nc.tensor.matmul(
                        h1p[:], lhsT=w1bt,
                        rhs=ne_b.to_broadcast([128, 2, CH]),
                        start=False, stop=True,
                    )
                    h1 = h1_pool.tile([128, 2, CH], bf16)
                    nc.scalar.activation(h1[:], h1p[:], AF.Relu, bias=b1[:])
                    h2p = psm2.tile([128, 2, CH], f32, tag="h2p")
                    nc.tensor.matmul(
                        h2p[:], lhsT=w2t, rhs=h1[:], start=True, stop=True
                    )
                    nvt = nvn_pool.tile([128, 2, CH], bf16)
                    nc.scalar.activation(nvt[:], h2p[:], AF.Identity,
                                         bias=b2[:])
                    r = nc.vector.tensor_add(nvt[:], nvt[:], pv2)  # residual
                    if cprep:
                        r._wait_ge(dma_sem, gtarget)
                    nvns.append(nvt)

                # ---- fused out-projection: m = nvn * woutT; ones-reduce
                wT = newt_sb[:, 1:2, :]
                m_all = m_pool.tile([128, BL, CH], bf16)
                pso_ts = []
                for t in range(2):
                    pso_t = pso.tile([128, 2, CH], f32, tag=f"pso{t}")
                    pso_ts.append(pso_t)
                for bp in range(BL // 2):
                    nc.vector.tensor_mul(
                        m_all[:, 2 * bp : 2 * bp + 2, :],
                        nvns[bp][:],
                        wT.to_broadcast([128, 2, CH]),
                    )
                    po = 32 * (bp % 2)
                    nc.tensor.matmul(
                        pso_ts[bp // 2][po : po + 1, :, :],
                        lhsT=ones_col,
                        rhs=m_all[:, 2 * bp : 2 * bp + 2, :],
                        start=True, stop=True,
                    )
                outsb = outsb_pool.tile([128, 2, 2, CH], f32)
                for t in range(2):
                    nc.scalar.copy(out=outsb[0:64, t, :, :],
                                   in_=pso_ts[t][0:64, :, :])
                    nc.sync.dma_start(out=outd[d, c, 2 * t : 2 * t + 2],
                                      in_=outsb[0:64:32, t, :, :])

                # ---- transpose back (bf16) and write token rows
                for kb in range(KB):
                    tp = pstp.tile([128, BL, 128], bf16, tag="tp")
                    for b in range(BL):
                        nc.tensor.transpose(
                            tp[:, b, :],
                            nvns[b // 2][:, b % 2, kb * 128 : (kb + 1) * 128],
                            identb,
                        )
                    nvrm = nvrm_pool.tile([128, BL, 128], bf16)
                    nc.vector.tensor_copy(out=nvrm[:], in_=tp[:])
                    tokbase = 2 + d * PP + c * CH + kb * 128
                    wnv = nc.sync.dma_start(
                        out=nv[tokbase : tokbase + 128, :],
                        in_=nvrm[:].rearrange("p b h -> p (b h)"),
                    )
                    nv_writes.setdefault(d, []).append(wnv.ins.name)

    nc.finalize()
    return nc


def _run_cores(nc, prep, embedding, n_cores):
    from concourse import bass_utils

    in_maps = []
    base = {
        "pidx": prep["pidx"],
        "newt": prep["newt"],
        "wcat": prep["wcat"],
        "b1c": prep["b1"],
        "b2c": prep["b2"],
    }
    for core in range(n_cores):
        eb = embedding[core * BL : (core + 1) * BL]  # [BL, H]
        nvinit = np.zeros((2, BL * H), np.float32)
        nvinit[1] = eb.reshape(-1)
        m = dict(base)
        m["nvinit"] = np.ascontiguousarray(nvinit.astype(BF16))
        in_maps.append(m)
    res = bass_utils.run_bass_kernel_spmd(
        nc, in_maps, core_ids=list(range(n_cores))
    )
    global LAST_RESULTS
    LAST_RESULTS = res
    return res


def _assemble(results, prep, inputs, D, P, CH, NCH, n_cores):
    embedding = np.asarray(inputs["embedding"], np.float32)
    Wout = np.asarray(inputs["Wout"], np.float32)
    bout = np.asarray(inputs["bout"], np.float32)
    NTOT = 1 + D * P

    out = np.empty((embedding.shape[0], NTOT), np.float32)
    out[:, 0] = embedding @ Wout[0] + bout[0]
    for core in range(n_cores):
        v = results[core]["outd"]  # [D, NCH, 4, 2, CH]
        for d in range(D):
            sg = np.concatenate(
                [v[d, c].reshape(BL, CH) for c in range(NCH)], axis=1
            )  # [BL, PP], col = sorted position
            cols = 1 + d * P + prep["perms"][d]
            out[core * BL : (core + 1) * BL, cols] = sg[:, :P]
    out[:, 1:] += bout[None, 1:]
    return out


def kernel(**inputs):
    D, P, CH, NCH = D_FULL, P_FULL, 256, 4
    prep = _prepare(inputs, D, P, CH, NCH)
    nc = _build(prep, D, P, CH, NCH)
    res = _run_cores(nc, prep, np.asarray(inputs["embedding"], np.float32), NCORES)
    return _assemble(res.results, prep, inputs, D, P, CH, NCH, NCORES)


# revision 34
# speedup vs baseline: 1.0254x; 1.0254x over previous
"""Trainium2 Bass kernel for nn_DAGModel (gnn_message_passing).

Strategy (data-parallel over batch, 8 b's per core):
- node_vecs live in DRAM as a bf16 table `nv[token, b8, h128]` (2KB rows).
- Parent gathers use GPSIMD dma_gather(transpose=True) in PREPARE_ONLY
  mode + trigger_dma: the gpsimd only generates descriptors (~1.1us) and
  the 2KB-row transfers run asynchronously on the DMA engines, so the
  Pool engine is no longer serialized on gather transfer time.
- Nodes of each depth are reordered host-side by (has-parent-in-previous-
  depth, parent-count desc). Chunks whose parents all come from older
  depths bound their gather source AP below the previous depth's slab, so
  those gathers overlap the previous depth's tail compute/writeback.
- The parent-slot sum accumulates IN PLACE into the slot-0 segment of the
  gathered tile (bf16 adds on DVE), which doubles as the MLP rhs `pv`.
- The 2-layer MLP runs in bf16 on the PE (f32 PSUM accumulate); ReLU+b1
  and +b2 ride the Scalar engine's activation; the residual +pv is a DVE
  bf16 add.
- Output projection out[t] = nv[t]·Wout[t] is computed feature-major:
  m = nvn * woutT elementwise (DVE, bf16), then a ones-vector matmul
  reduces over partitions into PSUM rows (per b-pair at partition 32*bp).
- new vecs are PE-transposed (bf16) back to row-major and DMA'd to the
  next depth's token rows.
"""

import numpy as np
import ml_dtypes

BF16 = ml_dtypes.bfloat16

# Full-problem dims (hardcoded per contract).
B, H, E = 64, 128, 128
D_FULL, P_FULL, MP = 20, 1000, 8
NCORES, BL = 8, 8
BCAP = 512  # SWDGE ring is ~512 descs/dir
LAST_RESULTS = None


# ---------------------------------------------------------------------------
# workaround: this walrus build rejects >1 sync-wait on a CTRL (Drain) inst.
def _install_tilefix():
    import concourse.tile as tile_mod
    from concourse.vector_clock import ScopedClock, VectorClock

    if getattr(tile_mod.TileContext, "_drain_split_installed", False):
        return

    def _split_drain_and_barrier(self, tick_clock, wait_clock):
        gc = tick_clock.global_clock
        ticks = list(gc)
        nz = [(i, t) for i, t in enumerate(ticks) if t > 0]
        if nz:
            for i, t in nz:
                vec = [0] * len(ticks)
                vec[i] = t
                d = self.nc.sync.drain()
                wait_clock.add_sem_waits(
                    d.ins, ScopedClock({None: VectorClock(vec)})
                )
        else:
            d = self.nc.sync.drain()
            wait_clock.add_sem_waits(d.ins, ScopedClock({None: gc}))
        self.nc.all_engine_barrier()
        assert self.sems is not None
        popped = self.nc._tile_sem_poison_stack.pop()
        assert popped is self._sem_poison
        self.nc.clear_and_free_semaphores(list(self.sems.allocated().values()))
        self.nc.all_engine_barrier()

    tile_mod.TileContext._drain_and_barrier = _split_drain_and_barrier
    tile_mod.TileContext._drain_split_installed = True


def _install_usersync_prep():
    """Route gen_mode==1 SWDGE gather preps onto their ENGINE proc
    (user-synced protocol) instead of a DMASW lane: Tile's DMASW-lane
    path for preps emits a pre-bumped doorbell + mismatched completion
    sem and deadlocks/races on this build. With the engine tick also
    registered in prep_eng_ticks, pass 2 gates trigger_dma on desc-gen
    completion; data completion is via the caller's sem= semaphore and
    explicit _wait_ge on consumer instructions."""
    import concourse.tile_sem_assignment as tsa
    import concourse.mybir as mybir

    if getattr(tsa.TileClockTick, "_usersync_prep_installed", False):
        return
    orig = tsa.TileClockTick._assign_tick

    def patched(self, inst):
        if getattr(inst, "gen_mode", 0) == 1 and isinstance(
            inst, (mybir.InstDMAGatherAnt, mybir.InstDMAScatterAddAnt)
        ):
            eng_proc_idx = (
                tsa.ENGINE_SEQUENCER_TO_IDX
                if inst.is_sequencer_only()
                else tsa.ENGINE_TO_IDX
            )[inst.engine]
            tick = self.global_clock.advance(eng_proc_idx)
            inst.bass_scheduled_tick = tick
            inst.bass_scheduled_proc = eng_proc_idx
            inst.bass_scheduled_scope = self.scope_name
            self._proc_insts[self.root_scope_name][eng_proc_idx].append(inst)
            self.tc.prep_eng_ticks[inst.name] = (eng_proc_idx, tick)
            self._prep_eng_names[self.root_scope_name].append(inst.name)
            return
        return orig(self, inst)

    tsa.TileClockTick._assign_tick = patched
    tsa.TileClockTick._usersync_prep_installed = True


# ---------------------------------------------------------------------------
def _wrap_idx(seq):
    """int16 index layout for dma_gather: position i -> [i%16, i//16],
    replicated across the 8 groups of 16 partitions."""
    a = np.asarray(seq, np.int16)
    L = len(a)
    assert L % 16 == 0
    a16 = a.reshape(L // 16, 16).T  # [16, L/16]
    return np.ascontiguousarray(np.tile(a16, (8, 1)))  # [128, L/16]


def _prepare(inputs, D, P, CH, NCH):
    """Host-side index preprocessing + weight/emb staging (layout only)."""
    PP = CH * NCH
    node_indices = np.asarray(inputs["node_indices"])
    parent_indices = np.asarray(inputs["parent_indices"])
    k = (parent_indices > 0).sum(-1)  # [D, P]
    recent = np.zeros((D, P), bool)
    for d in range(1, D):
        recent[d] = (parent_indices[d] >= 2 + (d - 1) * P).any(-1)

    remap = np.zeros(2 + D * P, np.int64)
    remap[1] = 1
    perms = []
    for d in range(D):
        # old-parent nodes first, then by parent count desc (prefix trick)
        perm = np.lexsort((-k[d], recent[d]))
        perms.append(perm)
        remap[2 + d * P + perm] = 2 + d * PP + np.arange(P)

    chunk_meta = []  # [d][c] -> dict(blens, adds, bound, w16)
    pidx_chunks = {}
    MAXW16 = 0
    MAXB = 0
    for d in range(D):
        perm = perms[d]
        row = []
        for c in range(NCH):
            lo = c * CH
            hi = min(lo + CH, P)
            nodes = perm[lo:hi]
            nreal = len(nodes)
            kc = k[d][nodes]
            rc = recent[d][nodes]
            og = int((~rc).sum())
            if d == 0:
                bound = 2
            elif og == nreal:
                bound = 2 + (d - 1) * PP
            else:
                bound = 2 + d * PP

            # segments: (col_start, idx_array); slot0 first (becomes pv)
            segs = [(0, np.pad(remap[parent_indices[d, nodes, 0]],
                               (0, CH - nreal)))]
            for j in range(1, MP):
                mo = int((kc[:og] > j).sum())
                if mo:
                    segs.append((0, remap[parent_indices[d, nodes[:mo], j]]))
                mr = int((kc[og:] > j).sum())
                if mr:
                    segs.append(
                        (og, remap[parent_indices[d, nodes[og:og + mr], j]]))

            # pack segments into <=BCAP bundles; record add ops
            blens, adds = [], []
            wrapped = []
            cur, cur_len = [], 0

            def close():
                nonlocal cur, cur_len
                if not cur_len:
                    return
                idx = np.concatenate(cur)
                L = (len(idx) + 127) // 128 * 128
                idx = np.pad(idx, (0, L - len(idx)))
                wrapped.append(_wrap_idx(idx))
                blens.append(L)
                cur, cur_len = [], 0

            for si, (cs, arr) in enumerate(segs):
                L = len(arr)
                if cur_len + L > BCAP and cur_len > 0:
                    close()
                if si > 0:
                    adds.append((len(blens), cur_len, cs, L))
                cur.append(arr)
                cur_len += L
            close()

            w16 = sum(L // 16 for L in blens)
            pidx_chunks[(d, c)] = np.concatenate(wrapped, axis=1)
            MAXW16 = max(MAXW16, w16)
            MAXB = max(MAXB, len(blens))
            row.append({"blens": blens, "adds": adds, "bound": bound,
                        "w16": w16})
        chunk_meta.append(row)

    pidx_np = np.zeros((D, NCH, 128, MAXW16), np.int16)
    for (d, c), w in pidx_chunks.items():
        pidx_np[d, c, :, : w.shape[1]] = w

    emb = np.asarray(inputs["emb_table"], np.float32)
    Wout = np.asarray(inputs["Wout"], np.float32)
    W1 = np.asarray(inputs["W1"], np.float32)
    W2 = np.asarray(inputs["W2"], np.float32)

    # per-chunk feature-major emb rows + Wout rows: [D, NCH, h, 2, CH]
    newt = np.zeros((D, NCH, H, 2, CH), np.float32)
    for d in range(D):
        for c in range(NCH):
            lo = c * CH
            hi = min(lo + CH, P)
            nodes = perms[d][lo:hi]
            newt[d, c, :, 0, : hi - lo] = emb[node_indices[d][nodes]].T
            newt[d, c, :, 1, : hi - lo] = Wout[1 + d * P + nodes].T

    wcat = np.zeros((128, 5, 128), np.float32)
    wcat[:, 0] = W1[:, :H].T
    wcat[:, 1] = W1[:, H:].T
    wcat[:, 2] = W2.T
    wcat[:, 3] = np.eye(128)
    wcat[:, 4, 0] = 1.0  # ones column for the partition-reduce matmul

    prep = {
        "meta": chunk_meta,
        "perms": perms,
        "maxw16": MAXW16,
        "pidx": pidx_np,
        "newt": np.ascontiguousarray(newt.astype(BF16)),
        "wcat": np.ascontiguousarray(wcat.astype(BF16)),
        "b1": np.asarray(inputs["b1"], np.float32).reshape(128, 1),
        "b2": np.asarray(inputs["b2"], np.float32).reshape(128, 1),
    }
    return prep


def _build(prep, D, P, CH, NCH):
    """Trace the Bass/Tile kernel. Returns a finalized Bacc."""
    import os
    PREP = os.environ.get("KPREP", "1") == "1"
    PREPN = int(os.environ.get("KPREPN", "99999"))
    _install_tilefix()
    if PREP:
        _install_usersync_prep()
    from contextlib import ExitStack

    import concourse.bacc as bacc
    import concourse.mybir as mybir
    from concourse.tile import TileContext

    PP = CH * NCH
    TOK = 2 + D * PP
    ROW = BL * H  # nv row elems (bf16)
    KB = CH // 128
    f32 = mybir.dt.float32
    bf16 = mybir.dt.bfloat16
    i16 = mybir.dt.int16
    AF = mybir.ActivationFunctionType

    nc = bacc.Bacc("TRN2", target_bir_lowering=False, debug=False)

    nv = nc.dram_tensor("nv", [TOK, ROW], bf16, kind="Internal")
    nvinit = nc.dram_tensor("nvinit", [2, ROW], bf16, kind="ExternalInput")
    pidx_in = nc.dram_tensor(
        "pidx", list(prep["pidx"].shape), i16, kind="ExternalInput"
    )
    newt_in = nc.dram_tensor(
        "newt", [D, NCH, 128, 2, CH], bf16, kind="ExternalInput"
    )
    wcat_in = nc.dram_tensor("wcat", [128, 5, 128], bf16, kind="ExternalInput")
    b1_in = nc.dram_tensor("b1c", [128, 1], f32, kind="ExternalInput")
    b2_in = nc.dram_tensor("b2c", [128, 1], f32, kind="ExternalInput")
    outd = nc.dram_tensor("outd", [D, NCH, 4, 2, CH], f32,
                          kind="ExternalOutput")

    meta = prep["meta"]

    with TileContext(nc) as tc, ExitStack() as ctx:
        const = ctx.enter_context(tc.tile_pool(name="const", bufs=1))
        pidx_pool = ctx.enter_context(tc.tile_pool(name="pidx", bufs=3))
        stag_pool = ctx.enter_context(tc.tile_pool(name="stag", bufs=3))
        newt_pool = ctx.enter_context(tc.tile_pool(name="newt", bufs=3))
        h1_pool = ctx.enter_context(tc.tile_pool(name="h1", bufs=3))
        nvn_pool = ctx.enter_context(tc.tile_pool(name="nvn", bufs=8))
        m_pool = ctx.enter_context(tc.tile_pool(name="m", bufs=2))
        nvrm_pool = ctx.enter_context(tc.tile_pool(name="nvrm", bufs=3))
        outsb_pool = ctx.enter_context(tc.tile_pool(name="outsb", bufs=2))
        psmm = ctx.enter_context(tc.tile_pool(name="psmm", bufs=2, space="PSUM"))
        psm2 = ctx.enter_context(tc.tile_pool(name="psm2", bufs=2, space="PSUM"))
        pstp = ctx.enter_context(tc.tile_pool(name="pstp", bufs=2, space="PSUM"))
        pso = ctx.enter_context(tc.tile_pool(name="pso", bufs=1, space="PSUM"))

        # rotation depth must be >= stag pool bufs+1 so same-sem chunks
        # can never have gathers in flight concurrently
        dma_sems = [nc.alloc_semaphore(f"swdma{i}") for i in range(4)]
        gath_cnt = [0, 0, 0, 0]

        wcat = const.tile([128, 5, 128], bf16)
        nc.sync.dma_start(out=wcat[:], in_=wcat_in[:, :, :])
        w1at = wcat[:, 0, :]
        w1bt = wcat[:, 1, :]
        w2t = wcat[:, 2, :]
        identb = wcat[:, 3, :]
        ones_col = wcat[:, 4, 0:1]
        b1 = const.tile([128, 1], f32)
        nc.sync.dma_start(out=b1[:], in_=b1_in[:, :])
        b2 = const.tile([128, 1], f32)
        nc.sync.dma_start(out=b2[:], in_=b2_in[:, :])

        # init nv rows 0..1 (zero pad row + root = per-b embedding)
        import bass_rust as _br
        _DI = _br.DependencyInfo(sync=True, no_sync=False)
        nv_writes = {}  # depth -> [dma inst names]
        nvi = const.tile([2, ROW], bf16)
        nc.sync.dma_start(out=nvi[:], in_=nvinit[:, :])
        winit = nc.sync.dma_start(out=nv[0:2, :], in_=nvi[:])
        nv_writes[-1] = [winit.ins.name]

        nchunk = 0
        for d in range(D):
            for c in range(NCH):
                cprep = PREP and nchunk < PREPN
                par = nchunk % 4
                dma_sem = dma_sems[par]
                nchunk += 1
                md = meta[d][c]
                blens, adds, bound = md["blens"], md["adds"], md["bound"]

                pidx_sb = pidx_pool.tile([128, md["w16"]], i16)
                nc.sync.dma_start(
                    out=pidx_sb[:], in_=pidx_in[d, c, :, : md["w16"]]
                )
                newt_sb = newt_pool.tile([128, 2, CH], bf16)
                nc.sync.dma_start(out=newt_sb[:], in_=newt_in[d, c])

                # source-writer sync deps go on the first PREP (the
                # trigger cannot carry >1 sem wait on this build)
                if d == 0:
                    wdeps = nv_writes[-1]
                elif bound == 2 + (d - 1) * PP:
                    wdeps = nv_writes.get(d - 2, nv_writes[-1])
                else:
                    wdeps = nv_writes[d - 1]
                stags = []
                off16 = 0
                for bi, L in enumerate(blens):
                    stag = stag_pool.tile([128, BL, L], bf16, tag=f"stag{bi}")
                    if cprep:
                        g = nc.gpsimd.dma_gather(
                            stag[:], nv[0:bound, :],
                            pidx_sb[:, off16 : off16 + L // 16],
                            num_idxs=L, num_idxs_reg=L,
                            elem_size=ROW, transpose=True,
                            prepare_only=True, sem=dma_sem,
                        )
                        for wn in wdeps:
                            g.ins.add_dependency(wn, _DI)
                        nc.gpsimd.trigger_dma(count=None)
                    else:
                        nc.gpsimd.dma_gather(
                            stag[:], nv[0:bound, :],
                            pidx_sb[:, off16 : off16 + L // 16],
                            num_idxs=L, num_idxs_reg=L,
                            elem_size=ROW, transpose=True,
                        )
                    off16 += L // 16
                    stags.append(stag)
                if cprep:
                    gath_cnt[par] += len(blens)
                gtarget = 16 * gath_cnt[par]

                # ---- parent-slot reduction in place into slot-0 segment
                pv = stags[0]
                for (bi, off, cs, L) in adds:
                    a = nc.vector.tensor_add(
                        pv[:, :, cs : cs + L],
                        pv[:, :, cs : cs + L],
                        stags[bi][:, :, off : off + L],
                    )
                    if cprep:
                        a._wait_ge(dma_sem, gtarget)

                # ---- MLP (bf16) over col pairs (2 b's x CH = 512 cols)
                ne_b = newt_sb[:, 0:1, :]
                nvns = []
                for bp in range(BL // 2):
                    pv2 = pv[:, 2 * bp : 2 * bp + 2, 0:CH]
                    h1p = psmm.tile([128, 2, CH], f32, tag="h1p")
                    mm = nc.tensor.matmul(
                        h1p[:], lhsT=w1at, rhs=pv2, start=True, stop=False
                    )
                    if cprep:
                        mm._wait_ge(dma_sem, gtarget)
                    nc.tensor.matmul(
                        h1p[:], lhsT=w1bt,
                        rhs=ne_b.to_broadcast([128, 2, CH]),
                        start=False, stop=True,
                    )
                    h1 = h1_pool.tile([128, 2, CH], bf16)
                    nc.scalar.activation(h1[:], h1p[:], AF.Relu, bias=b1[:])
                    h2p = psm2.tile([128, 2, CH], f32, tag="h2p")
                    nc.tensor.matmul(
                        h2p[:], lhsT=w2t, rhs=h1[:], start=True, stop=True
                    )
                    nvt = nvn_pool.tile([128, 2, CH], bf16)
                    nc.scalar.activation(nvt[:], h2p[:], AF.Identity,
                                         bias=b2[:])
                    r = nc.vector.tensor_add(nvt[:], nvt[:], pv2)  # residual
                    if cprep:
                        r._wait_ge(dma_sem, gtarget)
                    nvns.append(nvt)

                # ---- fused out-projection: m = nvn * woutT; ones-reduce
                wT = newt_sb[:, 1:2, :]
                m_all = m_pool.tile([128, BL, CH], bf16)
                pso_ts = []
                for t in range(2):
                    pso_t = pso.tile([128, 2, CH], f32, tag=f"pso{t}")
                    pso_ts.append(pso_t)
                for bp in range(BL // 2):
                    nc.vector.tensor_mul(
                        m_all[:, 2 * bp : 2 * bp + 2, :],
                        nvns[bp][:],
                        wT.to_broadcast([128, 2, CH]),
                    )
                    po = 32 * (bp % 2)
                    nc.tensor.matmul(
                        pso_ts[bp // 2][po : po + 1, :, :],
                        lhsT=ones_col,
                        rhs=m_all[:, 2 * bp : 2 * bp + 2, :],
                        start=True, stop=True,
                    )
                outsb = outsb_pool.tile([128, 2, 2, CH], f32)
                for t in range(2):
                    nc.scalar.copy(out=outsb[0:64, t, :, :],
                                   in_=pso_ts[t][0:64, :, :])
                    nc.sync.dma_start(out=outd[d, c, 2 * t : 2 * t + 2],
                                      in_=outsb[0:64:32, t, :, :])

                # ---- transpose back (bf16) and write token rows
                for kb in range(KB):
                    tp = pstp.tile([128, BL, 128], bf16, tag="tp")
                    for b in range(BL):
                        nc.tensor.transpose(
                            tp[:, b, :],
                            nvns[b // 2][:, b % 2, kb * 128 : (kb + 1) * 128],
                            identb,
                        )
                    nvrm = nvrm_pool.tile([128, BL, 128], bf16)
                    nc.vector.tensor_copy(out=nvrm[:], in_=tp[:])
                    tokbase = 2 + d * PP + c * CH + kb * 128
                    wnv = nc.sync.dma_start(
                        out=nv[tokbase : tokbase + 128, :],
                        in_=nvrm[:].rearrange("p b h -> p (b h)"),
                    )
                    nv_writes.setdefault(d, []).append(wnv.ins.name)

    nc.finalize()
    return nc


def _run_cores(nc, prep, embedding, n_cores):
    from concourse import bass_utils

    in_maps = []
    base = {
        "pidx": prep["pidx"],
        "newt": prep["newt"],
        "wcat": prep["wcat"],
        "b1c": prep["b1"],
        "b2c": prep["b2"],
    }
    for core in range(n_cores):
        eb = embedding[core * BL : (core + 1) * BL]  # [BL, H]
        nvinit = np.zeros((2, BL * H), np.float32)
        nvinit[1] = eb.reshape(-1)
        m = dict(base)
        m["nvinit"] = np.ascontiguousarray(nvinit.astype(BF16))
        in_maps.append(m)
    res = bass_utils.run_bass_kernel_spmd(
        nc, in_maps, core_ids=list(range(n_cores))
    )
    global LAST_RESULTS
    LAST_RESULTS = res
    return res


def _assemble(results, prep, inputs, D, P, CH, NCH, n_cores):
    embedding = np.asarray(inputs["embedding"], np.float32)
    Wout = np.asarray(inputs["Wout"], np.float32)
    bout = np.asarray(inputs["bout"], np.float32)
    NTOT = 1 + D * P

    out = np.empty((embedding.shape[0], NTOT), np.float32)
    out[:, 0] = embedding @ Wout[0] + bout[0]
    for core in range(n_cores):
        v = results[core]["outd"]  # [D, NCH, 4, 2, CH]
        for d in range(D):
            sg = np.concatenate(
                [v[d, c].reshape(BL, CH) for c in range(NCH)], axis=1
            )  # [BL, PP], col = sorted position
            cols = 1 + d * P + prep["perms"][d]
            out[core * BL : (core + 1) * BL, cols] = sg[:, :P]
    out[:, 1:] += bout[None, 1:]
    return out


def kernel(**inputs):
    D, P, CH, NCH = D_FULL, P_FULL, 256, 4
    prep = _prepare(inputs, D, P, CH, NCH)
    nc = _build(prep, D, P, CH, NCH)
    res = _run_cores(nc, prep, np.asarray(inputs["embedding"], np.float32), NCORES)
    return _assemble(res.results, prep, inputs, D, P, CH, NCH, NCORES)
